# revision 20
# baseline (speedup 1.0000x reference)
"""Trainium2 Bass kernel for nn_FMatrixLayer (bounded-recurrence matrix layer).

Algorithm notes
---------------
The reference scatters x*(N+1) into the deep lower triangle (r >= c+2) of an
NxN matrix F (diag = i+2, subdiag = c+1 fixed), then runs a sequential
row/column recurrence clamping each deep-triangle cell into bounds derived
from its left / upper / diagonal neighbors, accumulating loss = sum of
clamp corrections.  Output = corrected triangle / (N+1), plus scalar loss.

Cell (r, c) depends on (r, c-1), (r-1, c), (r-1, c-1)  ->  anti-diagonals
t = r + c form a wavefront: ~2N-5 sequential steps, cells within a step are
independent.

Device layout (per core): batch on the 128 partitions, wavefront cells on
the free dim.  All of x is host-permuted into diagonal-major order
("xw" = [128 partitions, G=2 batch groups, L slots]) so that every
neighbor read is a constant free-dim offset.  Each diagonal's slot range is
[pad, active cells jlo..jhi, (subdiag const)]:
  * pad slot (value LOW) makes the column-0 boundary case fall out of the
    generic formula exactly: left = dg = LOW  =>  (left - dg) + up == up.
  * subdiag slots hold the fixed value c+1 = (t+1)/2 so `up` reads of row
    i-1's subdiagonal resolve uniformly.
Cells are updated in place (new value overwrites curr in the same slot);
later diagonals read the updated values at fixed offsets.

Per step (fused via scalar_tensor_tensor, all in F-space):
  t1  = left - dg
  l3s = t1 + up                      # == left + up - dg
  f1  = (up - 1) max left
  f2  = (l3s - 1) max f1             # = max(left, up-1, l3s-1);  fl = max(f2, 0)
  m   = up min l3s
  fu  = (m max 0) max f2             # = max(min(up, l3s), fl)
  n1  = curr min fu
  new = (n1 max 0) max f2            # = clamp(curr, fl, fu)
loss is recovered at the end per chunk:  sum |X_final - X_orig|  (inactive
slots are untouched so they contribute exactly 0), accumulated per
partition with ACT Abs+accum_out and finished on the host in float64.

Sharding: pure data parallel, batch 2048 = 8 cores x (2 groups x 128
partitions).  No collectives; the scalar loss partials are summed on host.
"""

import sys

for _p in ("/opt/trn_rl_repo",):
    if _p not in sys.path:
        sys.path.append(_p)

import numpy as np

import concourse.bacc as bacc
import concourse.mybir as mybir
from concourse import tile
from concourse.alu_op_type import AluOpType as Op
from concourse.bass_utils import run_bass_kernel_spmd

N = 128
BATCH = 2048
INPUT_DIM = (N - 1) * (N - 2) // 2
NCORES = 8
P = 128                    # SBUF partitions = batch tile
G = BATCH // (NCORES * P)  # batch groups per core (= 2)
SCALE = float(N + 1)
LOW = -1.0e9
NCHUNK = 16
F32 = mybir.dt.float32
ABS = mybir.ActivationFunctionType.Abs


def _layout(n=N):
    """Diagonal-major slot layout. Returns dict of per-diag tables + cell map."""
    tmax = 2 * n - 4                       # last diag with an active cell
    jlo = np.maximum(0, np.arange(tmax + 1) - (n - 1))
    jhi = (np.arange(tmax + 1) - 2) // 2
    wa = np.maximum(0, jhi - jlo + 1)      # active cells per diag
    wa[:2] = 0
    has_sub = (np.arange(tmax + 1) % 2) == 1
    width = 1 + wa + has_sub.astype(np.int64)
    off = np.concatenate([[0], np.cumsum(width)[:-1]])
    L = int(width.sum())

    # active cell (r, c), reference flat order: for c: for r in c+2..n-1
    rows, cols = [], []
    for c in range(n):
        for r in range(c + 2, n):
            rows.append(r)
            cols.append(c)
    rows = np.asarray(rows, np.int64)
    cols = np.asarray(cols, np.int64)
    t_of = rows + cols
    pos = off[t_of] + 1 + (cols - jlo[t_of])

    # prefill template: pads LOW, subdiags (t+1)/2, rest 0 (overwritten / unread)
    base = np.zeros(L, np.float32)
    base[off] = LOW
    ts = np.arange(tmax + 1)
    sub_t = ts[has_sub]
    base[off[sub_t] + 1 + wa[sub_t]] = (sub_t + 1) / 2.0

    # per-step slice origins (global slot offsets); yoff = read offset into
    # the previous step's Y ring tile (position 0 there is the zero pad)
    steps = []
    for t in range(2, tmax + 1):
        w = int(wa[t])
        if w == 0:
            continue
        curr = int(off[t] + 1)
        left = int(off[t - 1] + 1 + (jlo[t] - 1 - jlo[t - 1]))
        yoff = int(jlo[t] - jlo[t - 1])
        assert off[t - 1] <= left and left + 1 + w <= off[t - 1] + width[t - 1], t
        assert 0 <= yoff <= 1, t
        steps.append((t, curr, left, left + 1, yoff, w))

    # chunk diag range [0, tmax] into diag-aligned groups ~equal width, but
    # with small head chunks (compute starts after chunk 0's DMA) and a
    # small tail chunk (its loss+output DMA is serial tail latency)
    k = min(NCHUNK, tmax + 1)
    cum = np.cumsum(width)
    head1, head2 = int(tmax * 0.10), int(tmax * 0.22)
    tail0 = tmax - max(1, int(tmax * 0.04))
    bounds = [0, head1, head2]
    target = (cum[tail0 - 1] - cum[head2 - 1]) / max(1, k - 4)
    acc = 0
    for t in range(head2, tail0):
        acc += int(width[t])
        if acc >= target * (len(bounds) - 2) and len(bounds) < k - 1:
            bounds.append(t + 1)
    bounds.append(tail0)
    bounds = sorted(set(b for b in bounds if b <= tmax)) + [tmax + 1]
    chunks = []            # (t_start, t_end, slot_a, slot_b)
    for i in range(len(bounds) - 1):
        t0, t1 = bounds[i], bounds[i + 1]
        a = int(off[t0])
        b = int(off[t1 - 1] + width[t1 - 1])
        chunks.append((t0, t1, a, b))
    chunk_of = np.zeros(tmax + 1, np.int64)
    for ci, (t0, t1, _, _) in enumerate(chunks):
        chunk_of[t0:t1] = ci

    return dict(n=n, tmax=tmax, jlo=jlo, jhi=jhi, wa=wa, width=width, off=off,
                L=L, pos=pos, base=base, steps=steps, chunks=chunks,
                chunk_of=chunk_of)


def _build(lay, scr_bufs=2, skip_loss=False, skip_out=False,
           loss_slices=4):
    """Build the single-core Bass program (SPMD across 8 cores)."""
    L = lay["L"]
    nc = bacc.Bacc(None, target_bir_lowering=False)
    xw = nc.dram_tensor("xw", [P, G, L], F32, kind="ExternalInput")
    yw = nc.dram_tensor("yw", [P, G, L], F32, kind="ExternalOutput")
    lacc_d = nc.dram_tensor("lacc", [P, len(lay["chunks"])], F32,
                            kind="ExternalOutput")

    chunks = lay["chunks"]
    chunk_of = lay["chunk_of"]
    steps_by_chunk = {ci: [] for ci in range(len(chunks))}
    for st in lay["steps"]:
        steps_by_chunk[int(chunk_of[st[0]])].append(st)

    with tile.TileContext(nc) as tc:
        with (
            tc.tile_pool(name="data", bufs=1) as dp,
            tc.tile_pool(name="scr", bufs=scr_bufs) as sp,
            tc.tile_pool(name="misc", bufs=1) as mp,
        ):
            X = [dp.tile([P, G, b - a], F32, tag=f"x{k}", name=f"x{k}")
                 for k, (_, _, a, b) in enumerate(chunks)]
            XO = [dp.tile([P, G, b - a], F32, tag=f"o{k}", name=f"o{k}")
                  for k, (_, _, a, b) in enumerate(chunks)]
            lacc = mp.tile([P, len(chunks)], F32, tag="lacc", name="lacc")
            # Y ring: column deltas of the last two diagonals; position 0 is
            # a permanent zero pad (makes col-0 fall out of the formula).
            YR = [mp.tile([P, G, 64], F32, tag=f"y{r}", name=f"y{r}")
                  for r in range(2)]
            nc.vector.memset(YR[0][:], 0.0)
            nc.vector.memset(YR[1][:], 0.0)

            def sl(t, start, w):
                ci = int(chunk_of[t])
                lo = start - chunks[ci][2]
                return X[ci][:, :, lo:lo + w]

            for ci, (_, _, a, b) in enumerate(chunks):
                nc.sync.dma_start(X[ci][:, :, :], xw[:, :, a:b])
                if not skip_loss:
                    nc.sync.dma_start(XO[ci][:, :, :], xw[:, :, a:b])

                for (t, c0, l0, u0, yo, w) in steps_by_chunk[ci]:
                    curr = sl(t, c0, w)
                    left = sl(t - 1, l0, w)
                    up = sl(t - 1, u0, w)
                    yp = YR[(t - 1) % 2][:, :, yo:yo + w]
                    yc = YR[t % 2][:, :, 1:1 + w]
                    # delta form (everything minus `up`); Y' = left - dg:
                    #   n'  = max(min(curr-up, 0, Y'), Y'-1, -1, left-up)
                    #   new = up + n';   Y_t = n'
                    cu = sp.tile([P, G, w], F32, tag="cu", name="cu")
                    lu = sp.tile([P, G, w], F32, tag="lu", name="lu")
                    m = sp.tile([P, G, w], F32, tag="m", name="m")
                    s = sp.tile([P, G, w], F32, tag="s", name="s")
                    nc.vector.tensor_tensor(cu[:], curr, up, Op.subtract)
                    nc.vector.tensor_tensor(lu[:], left, up, Op.subtract)
                    nc.vector.scalar_tensor_tensor(
                        m[:], cu[:], 0.0, yp, Op.min, Op.min)
                    nc.vector.scalar_tensor_tensor(
                        s[:], yp, 1.0, lu[:], Op.subtract, Op.max)
                    nc.vector.scalar_tensor_tensor(
                        yc, m[:], -1.0, s[:], Op.max, Op.max)
                    nc.vector.tensor_tensor(curr, up, yc, Op.add)

                if not skip_loss:
                    cw = b - a
                    d = sp.tile([P, G, cw], F32, tag="d", name="d")
                    ns = max(1, min(loss_slices, cw))
                    for si in range(ns):
                        s0 = si * cw // ns
                        s1 = (si + 1) * cw // ns
                        nc.gpsimd.tensor_tensor(
                            d[:, :, s0:s1], X[ci][:, :, s0:s1],
                            XO[ci][:, :, s0:s1], Op.subtract)
                    nc.scalar.activation(d[:], d[:], ABS,
                                         accum_out=lacc[:, ci:ci + 1])
                if not skip_out:
                    nc.sync.dma_start(yw[:, :, a:b], X[ci][:, :, :])

            if skip_out:   # keep yw an output: dump one chunk
                nc.sync.dma_start(yw[:, :, 0:chunks[0][3]], X[0][:, :, :])
            if skip_loss:
                nc.gpsimd.memset(lacc[:, :], 0.0)
            nc.sync.dma_start(lacc_d[:, :], lacc[:, :])
    nc.compile()
    return nc


_CACHE = {}


def _get(**kw):
    key = tuple(sorted(kw.items()))
    if key not in _CACHE:
        lay = _layout()
        _CACHE[key] = (lay, _build(lay, **kw))
    return _CACHE[key]


def make_inputs(x, lay):
    """Host permute: x (BATCH, INPUT_DIM) -> per-core xw [P, G, L] (scaled)."""
    x = np.ascontiguousarray(np.asarray(x, np.float32))
    xs = x.reshape(NCORES, G, P, INPUT_DIM).transpose(0, 2, 1, 3)  # [c,p,g,k]
    xw = np.broadcast_to(lay["base"], (NCORES, P, G, lay["L"])).copy()
    xw[:, :, :, lay["pos"]] = xs * np.float32(SCALE)
    return xw


def gather_outputs(results, lay):
    """Per-core {yw, lacc} -> (out (BATCH, INPUT_DIM) f32, loss f32)."""
    y = np.empty((NCORES, G, P, INPUT_DIM), np.float32)
    loss = 0.0
    for c in range(NCORES):
        ywc = np.asarray(results[c]["yw"]).reshape(P, G, lay["L"])
        y[c] = ywc[:, :, lay["pos"]].transpose(1, 0, 2)
        loss += np.asarray(results[c]["lacc"], np.float64).sum()
    out = (y.reshape(BATCH, INPUT_DIM) / np.float32(SCALE)).astype(np.float32)
    return out, np.float32(loss)


def kernel(x):
    lay, nc = _get()
    xw = make_inputs(x, lay)
    res = run_bass_kernel_spmd(
        nc, [{"xw": xw[c]} for c in range(NCORES)], list(range(NCORES)))
    return gather_outputs(res.results, lay)
